# revision 1
# baseline (speedup 1.0000x reference)
"""
Trainium2 Bass kernel for DirectRankingModel:
    h = tanh(x @ W1.T + b1); s = (h @ W2.T + b2); e = exp(s)
    out = e / segment_sum(e, T)[T]    with 2 segments, N = 2,000,000 rows.

Strategy (8 NeuronCores, data-parallel over rows):
  - Host: block-transpose x into [nblk, 64 feat, 128 rows] so each DMA
    descriptor moves contiguous 512B runs and the PE receives the
    feature-on-partition (transposed) operand directly.  Host also builds
    f32 masks m0/m1 = (T==0)/(T==1) (zero on padded rows).
  - Device per core (R = 262144 rows, padded; 8 super-tiles of 128x256):
      * SWDGE DMA with f32->f16 cast loads "xx" mega tiles [128, 2048]:
        partitions = (half, feature), free = rows.
      * mm1: two K=64 matmuls per 1024 rows (row-split PE: partitions 0-63
        and 64-127 run concurrently), W1T stationary -> PSUM hT [128h, 1024r].
      * tanh on the scalar engine with fused +b1 bias, PSUM -> SBUF fp16.
      * mm2: score s = W2 . h per row, laid out as [128 blocks, 256 rows]:
        32 strip matrices [128, 32] with W2 embedded in column c accumulate
        block b's scores into PSUM partition b (avoids a [1, N] layout).
      * exp with fused +b2 bias -> E [128, 2048] f32 stays SBUF-resident.
      * masked sums via tensor_tensor_reduce, partition_all_reduce, then a
        2-float HBM AllReduce across the 8 cores.
      * normalize: out = E * (inv0 + m1*(inv1-inv0)) and DMA out.
"""

import os
import sys

import numpy as np

for _p in ("/opt/trn_rl_repo", "/root/.axon_site/_ro/trn_rl_repo"):
    if os.path.isdir(_p) and _p not in sys.path:
        sys.path.insert(0, _p)

import concourse.bacc as bacc
import concourse.bass as bass
import concourse.tile as tile
from concourse import bass_isa, mybir
from concourse.bass_utils import run_bass_kernel_spmd

F16 = mybir.dt.float16
F32 = mybir.dt.float32
ALU = mybir.AluOpType
ACTF = mybir.ActivationFunctionType

N_CORES = 8
N_ROWS = 2_000_000
IN_DIM = 64
HID = 128

# Device-side geometry (per core).
Q = 256                 # rows per score-block (mm2 moving free dim)
N_ST = 8                # super-tiles per core; ST = 128 blocks x Q rows = 32768
MEGA_BLK = 32           # x blocks (128 rows each) per mega DMA tile -> 4096 rows
R_CORE = N_ST * 128 * Q          # 262144 rows per core
NBLK_CORE = R_CORE // 128        # 2048
N_PAD = N_CORES * R_CORE         # 2097152 rows total (padded)
NBLK_TOT = N_PAD // 128          # 16384
NBLK_REAL = N_ROWS // 128        # 15625

_MEGAS_PER_ST = (128 * Q) // (MEGA_BLK * 128)   # 8
_SUB_PER_MEGA = (MEGA_BLK * 128) // 1024        # 4  (1024-row mm1 pairs)
_BLOCKS_PER_MEGA = (MEGA_BLK * 128) // Q        # 16 (mm2 blocks per mega)


def _ap(handle_ap, offset, dims):
    """Custom access pattern on a DRAM tensor: dims = [[step, count], ...]."""
    return bass.AP(tensor=handle_ap.tensor, offset=offset, ap=list(dims))


def build_nc(n_st=N_ST, n_cores=N_CORES, use_coll=True, stage=9):
    """Build the per-core Bass program (SPMD: same program, sliced inputs)."""
    from contextlib import ExitStack

    r_core = n_st * 128 * Q
    nblk = r_core // 128
    cols = n_st * Q            # E/mask/out columns per partition

    n_mega = r_core // (MEGA_BLK * 128)

    nc = bacc.Bacc(num_devices=n_cores)

    x_in = nc.declare_dram_parameter(
        "x", [n_mega, 128, MEGA_BLK * 64], F32, isOutput=False
    )
    m0_in = nc.declare_dram_parameter("m0", [r_core], F32, isOutput=False)
    m1_in = nc.declare_dram_parameter("m1", [r_core], F32, isOutput=False)
    w1t_in = nc.declare_dram_parameter("w1t", [IN_DIM, HID], F16, isOutput=False)
    w2s_in = nc.declare_dram_parameter("w2s", [HID, 32 * 32], F16, isOutput=False)
    b1_in = nc.declare_dram_parameter("b1", [HID], F32, isOutput=False)
    b2_in = nc.declare_dram_parameter("b2", [1], F32, isOutput=False)
    out_t = nc.declare_dram_parameter("out", [r_core], F32, isOutput=True)
    gs_t = nc.declare_dram_parameter("gsums", [2], F32, isOutput=True)

    cc_in = nc.dram_tensor("cc_in", [2], F32)
    cc_out = nc.dram_tensor("cc_out", [2], F32, addr_space="Shared")

    B_ELEM = IN_DIM * 128  # elements per x block

    with ExitStack() as ctx:
        tc = ctx.enter_context(tile.TileContext(nc))
        singles = ctx.enter_context(tc.tile_pool(name="singles", bufs=1))
        xx_pool = ctx.enter_context(tc.tile_pool(name="xx", bufs=3))
        ht_pool = ctx.enter_context(tc.tile_pool(name="ht", bufs=3))
        ph_pool = ctx.enter_context(tc.tile_pool(name="ph", bufs=3, space="PSUM"))
        ps_pool = ctx.enter_context(tc.tile_pool(name="ps", bufs=1, space="PSUM"))

        # ---- static setup ----------------------------------------------
        w1t_sb = singles.tile([128, HID], F16)     # both halves hold W1T
        nc.sync.dma_start(
            out=w1t_sb[:],
            in_=_ap(w1t_in[:], 0, [[0, 2], [HID, IN_DIM], [1, HID]]),
        )
        b1_sb = singles.tile([128, 1], F32)
        nc.sync.dma_start(out=b1_sb[:], in_=_ap(b1_in[:], 0, [[1, HID], [1, 1]]))
        b2_sb = singles.tile([128, 1], F32)
        nc.sync.dma_start(out=b2_sb[:], in_=_ap(b2_in[:], 0, [[0, 128], [1, 1]]))

        # 32 strip matrices [128, 32] fp16, strip c has W2 in column c.
        strips = singles.tile([128, 32, 32], F16)
        nc.sync.dma_start(
            out=strips[:], in_=_ap(w2s_in[:], 0, [[32 * 32, HID], [1, 32 * 32]])
        )

        # Masks + persistent E (all f32, SBUF-resident for the whole kernel).
        m0_sb = singles.tile([128, cols], F32)
        m1_sb = singles.tile([128, cols], F32)
        mask_dims = [[Q, 128], [128 * Q, n_st], [1, Q]]
        nc.sync.dma_start(out=m0_sb[:], in_=_ap(m0_in[:], 0, mask_dims))
        nc.sync.dma_start(out=m1_sb[:], in_=_ap(m1_in[:], 0, mask_dims))
        e_sb = singles.tile([128, cols], F32)
        scratch = singles.tile([128, cols], F32)
        out_sb = singles.tile([128, cols], F32)
        rr = singles.tile([128, 2], F32)
        rr_red = singles.tile([128, 2], F32)
        ones_sb = singles.tile([128, 1], F32)
        nc.vector.memset(ones_sb[:], 1.0)
        g_sb = singles.tile([128, 2], F32)
        inv = singles.tile([128, 2], F32)
        dinv = singles.tile([128, 1], F32)

        # ---- phase 1: matmuls / tanh / scores / exp --------------------
        for st in range(n_st):
            s_ps = ps_pool.tile([128, Q], F32, tag="score")
            for m in range(_MEGAS_PER_ST):
                mega = st * _MEGAS_PER_ST + m
                half = MEGA_BLK * 64  # 2048 rows: partition halves g=0/1
                xx = xx_pool.tile([128, half], F16, tag="xx")
                src = _ap(
                    x_in[:],
                    mega * 128 * half,
                    [[half, 128], [1, half]],
                )
                nc.gpsimd.dma_start(out=xx[:], in_=src)  # f32 -> f16 cast DMA

                ht = ht_pool.tile([128, MEGA_BLK * 128], F16, tag="ht")
                for t in range(_SUB_PER_MEGA):
                    ph = ph_pool.tile([128, 1024], F32, tag="ph")
                    nc.tensor.matmul(
                        ph[:, 0:512],
                        w1t_sb[0:64, :],
                        xx[0:64, t * 512 : (t + 1) * 512],
                        start=True,
                        stop=True,
                    )
                    nc.tensor.matmul(
                        ph[:, 512:1024],
                        w1t_sb[64:128, :],
                        xx[64:128, t * 512 : (t + 1) * 512],
                        start=True,
                        stop=True,
                    )
                    # ht col layout is (t, g, j): col = t*1024 + g*512 + j,
                    # holding row mega_base + g*2048 + t*512 + j.
                    nc.scalar.activation(
                        out=ht[:, t * 1024 : (t + 1) * 1024],
                        in_=ph[:, 0:1024],
                        func=ACTF.Tanh,
                        bias=b1_sb[:],
                        scale=1.0,
                    )
                for bl in range(_BLOCKS_PER_MEGA):
                    b = m * _BLOCKS_PER_MEGA + bl
                    c = b % 32
                    g = b // 32
                    # rows bl*256..+256 of this mega live at ht col offset:
                    hoff = ((bl % 8) // 2) * 1024 + (bl // 8) * 512 + (bl % 2) * Q
                    nc.tensor.matmul(
                        s_ps[32 * g : 32 * g + 32, :],
                        strips[:, c, :],
                        ht[:, hoff : hoff + Q],
                        start=(c == 0),
                        stop=(c == 31),
                        skip_group_check=True,
                        tile_position=(0, 32 * g),
                    )
            nc.scalar.activation(
                out=e_sb[:, st * Q : (st + 1) * Q],
                in_=s_ps[:],
                func=ACTF.Exp,
                bias=b2_sb[:],
                scale=1.0,
            )

        # ---- segment sums + allreduce ----------------------------------
        if stage <= 1:
            # phase-1 only: dump E and a dummy gsums
            nc.sync.dma_start(
                out=_ap(out_t[:], 0, [[Q, 128], [128 * Q, n_st], [1, Q]]),
                in_=e_sb[:],
            )
            nc.sync.dma_start(out=gs_t[:], in_=e_sb[0:1, 0:2])
            nc.compile()
            return nc
        nc.vector.tensor_mul(scratch[:], e_sb[:], m0_sb[:])
        nc.vector.reduce_sum(rr[:, 0:1], scratch[:], axis=mybir.AxisListType.X)
        nc.vector.tensor_mul(scratch[:], e_sb[:], m1_sb[:])
        nc.vector.reduce_sum(rr[:, 1:2], scratch[:], axis=mybir.AxisListType.X)
        if stage <= 2:
            # skip partition reduce: use per-partition sums (wrong values)
            nc.vector.tensor_copy(rr_red[:], rr[:])
        else:
            # cross-partition sum via ones-matmul (PE), [128,2] -> [1,2]
            ps_rr = ps_pool.tile([128, 2], F32, tag="score")
            nc.tensor.matmul(
                ps_rr[0:1, :], ones_sb[:], rr[:], start=True, stop=True
            )
            nc.scalar.activation(
                out=rr_red[0:1, :],
                in_=ps_rr[0:1, :],
                func=ACTF.Copy,
                bias=0.0,
                scale=1.0,
            )
        if use_coll:
            nc.gpsimd.dma_start(out=cc_in[:], in_=rr_red[0:1, :])
            nc.gpsimd.collective_compute(
                "AllReduce",
                ALU.add,
                replica_groups=[list(range(n_cores))],
                ins=[cc_in[:]],
                outs=[cc_out[:]],
            )
            nc.sync.dma_start(out=gs_t[:], in_=cc_out[:])
            nc.sync.dma_start(
                out=g_sb[:], in_=_ap(cc_out[:], 0, [[0, 128], [1, 2]])
            )
        else:
            nc.sync.dma_start(out=gs_t[:], in_=rr_red[0:1, :])
            nc.vector.tensor_copy(g_sb[:], rr_red[:])

        # ---- normalize + store -----------------------------------------
        nc.vector.reciprocal(out=inv[:], in_=g_sb[:])
        nc.vector.tensor_sub(dinv[:], inv[:, 1:2], inv[:, 0:1])
        nc.vector.tensor_scalar(
            out=scratch[:],
            in0=m1_sb[:],
            scalar1=dinv[:],
            scalar2=inv[:, 0:1],
            op0=ALU.mult,
            op1=ALU.add,
        )
        nc.vector.tensor_mul(out_sb[:], scratch[:], e_sb[:])
        nc.sync.dma_start(
            out=_ap(out_t[:], 0, [[Q, 128], [128 * Q, n_st], [1, Q]]),
            in_=out_sb[:],
        )

    nc.compile()
    return nc


_NC_CACHE = {}


def _get_nc(n_st=N_ST):
    if n_st not in _NC_CACHE:
        _NC_CACHE[n_st] = build_nc(n_st=n_st)
    return _NC_CACHE[n_st]


def prep_inputs(x, T, W1, b1, W2, b2, n_st=N_ST, n_cores=N_CORES):
    """Host-side shard/layout prep -> per-core input maps."""
    r_core = n_st * 128 * Q
    nblk = r_core // 128
    n_pad = n_cores * r_core
    n_rows = x.shape[0]
    nblk_real = n_rows // 128

    x = np.ascontiguousarray(np.asarray(x, dtype=np.float32))
    rows_mega = MEGA_BLK * 128                      # 4096
    half = rows_mega // 2                           # 2048
    n_mega_tot = n_pad // rows_mega
    n_full = n_rows // rows_mega
    xd = np.zeros((n_mega_tot, 128, half), dtype=np.float32)
    xd[:n_full] = (
        x[: n_full * rows_mega]
        .reshape(n_full, 2, half, IN_DIM)
        .transpose(0, 1, 3, 2)
        .reshape(n_full, 128, half)
    )
    rem = n_rows - n_full * rows_mega
    if rem:
        r0 = min(rem, half)
        xd[n_full, :IN_DIM, :r0] = x[n_full * rows_mega :][:r0].T
        if rem > half:
            xd[n_full, IN_DIM:, : rem - half] = x[n_full * rows_mega + half :].T
    n_mega_core = n_mega_tot // n_cores

    T = np.asarray(T)
    m0 = np.zeros(n_pad, dtype=np.float32)
    m1 = np.zeros(n_pad, dtype=np.float32)
    m0[:n_rows] = T == 0
    m1[:n_rows] = T == 1

    w1t = np.ascontiguousarray(np.asarray(W1, np.float32).T).astype(np.float16)
    w2s = np.zeros((HID, 32, 32), dtype=np.float16)
    w2v = np.asarray(W2, np.float32).reshape(HID).astype(np.float16)
    for c in range(32):
        w2s[:, c, c] = w2v
    w2s = w2s.reshape(HID, 32 * 32)
    b1h = np.asarray(b1, np.float32).reshape(HID).copy()
    b2h = np.asarray(b2, np.float32).reshape(1).copy()

    in_maps = []
    for cid in range(n_cores):
        in_maps.append(
            {
                "x": xd[cid * n_mega_core : (cid + 1) * n_mega_core],
                "m0": m0[cid * r_core : (cid + 1) * r_core],
                "m1": m1[cid * r_core : (cid + 1) * r_core],
                "w1t": w1t,
                "w2s": w2s,
                "b1": b1h,
                "b2": b2h,
            }
        )
    return in_maps


def run(x, T, W1, b1, W2, b2, n_st=N_ST, trace=False):
    in_maps = prep_inputs(x, T, W1, b1, W2, b2, n_st=n_st)
    nc = _get_nc(n_st)
    res = run_bass_kernel_spmd(nc, in_maps, list(range(N_CORES)), trace=trace)
    out = np.concatenate([res.results[c]["out"] for c in range(N_CORES)])
    return out[: x.shape[0]].astype(np.float32, copy=False), res


def kernel(x, T, W1, b1, W2, b2):
    out, _ = run(x, T, W1, b1, W2, b2)
    return out



# revision 4
# speedup vs baseline: 1.3143x; 1.3143x over previous
"""
Trainium2 Bass kernel for DirectRankingModel:
    h = tanh(x @ W1.T + b1); s = (h @ W2.T + b2); e = exp(s)
    out = e / segment_sum(e, T)[T]    with 2 segments, N = 2,000,000 rows.

Strategy (8 NeuronCores, data-parallel over rows):
  - Host: block-transpose x into [nblk, 64 feat, 128 rows] so each DMA
    descriptor moves contiguous 512B runs and the PE receives the
    feature-on-partition (transposed) operand directly.  Host also builds
    f32 masks m0/m1 = (T==0)/(T==1) (zero on padded rows).
  - Device per core (R = 262144 rows, padded; 8 super-tiles of 128x256):
      * SWDGE DMA with f32->f16 cast loads "xx" mega tiles [128, 2048]:
        partitions = (half, feature), free = rows.
      * mm1: two K=64 matmuls per 1024 rows (row-split PE: partitions 0-63
        and 64-127 run concurrently), W1T stationary -> PSUM hT [128h, 1024r].
      * tanh on the scalar engine with fused +b1 bias, PSUM -> SBUF fp16.
      * mm2: score s = W2 . h per row, laid out as [128 blocks, 256 rows]:
        32 strip matrices [128, 32] with W2 embedded in column c accumulate
        block b's scores into PSUM partition b (avoids a [1, N] layout).
      * exp with fused +b2 bias -> E [128, 2048] f32 stays SBUF-resident.
      * masked sums via tensor_tensor_reduce, partition_all_reduce, then a
        2-float HBM AllReduce across the 8 cores.
      * normalize: out = E * (inv0 + m1*(inv1-inv0)) and DMA out.
"""

import os
import sys

import numpy as np

for _p in ("/opt/trn_rl_repo", "/root/.axon_site/_ro/trn_rl_repo"):
    if os.path.isdir(_p) and _p not in sys.path:
        sys.path.insert(0, _p)

import concourse.bacc as bacc
import concourse.bass as bass
import concourse.tile as tile
from concourse import bass_isa, mybir
from concourse.bass_utils import run_bass_kernel_spmd

F16 = mybir.dt.float16
F32 = mybir.dt.float32
ALU = mybir.AluOpType
ACTF = mybir.ActivationFunctionType

N_CORES = 8
N_ROWS = 2_000_000
IN_DIM = 64
HID = 128

# Device-side geometry (per core).
Q = 256                 # rows per score-block (mm2 moving free dim)
N_ST = 8                # super-tiles per core; ST = 128 blocks x Q rows = 32768
MEGA_BLK = 32           # x blocks (128 rows each) per mega DMA tile -> 4096 rows
R_CORE = N_ST * 128 * Q          # 262144 rows per core
NBLK_CORE = R_CORE // 128        # 2048
N_PAD = N_CORES * R_CORE         # 2097152 rows total (padded)
NBLK_TOT = N_PAD // 128          # 16384
NBLK_REAL = N_ROWS // 128        # 15625

_MEGAS_PER_ST = (128 * Q) // (MEGA_BLK * 128)   # 8
_SUB_PER_MEGA = (MEGA_BLK * 128) // 1024        # 4  (1024-row mm1 pairs)
_BLOCKS_PER_MEGA = (MEGA_BLK * 128) // Q        # 16 (mm2 blocks per mega)


def _ap(handle_ap, offset, dims):
    """Custom access pattern on a DRAM tensor: dims = [[step, count], ...]."""
    return bass.AP(tensor=handle_ap.tensor, offset=offset, ap=list(dims))


def build_nc(n_st=N_ST, n_cores=N_CORES, use_coll=True, stage=9):
    """Build the per-core Bass program (SPMD: same program, sliced inputs)."""
    from contextlib import ExitStack

    r_core = n_st * 128 * Q
    nblk = r_core // 128
    cols = n_st * Q            # E/mask/out columns per partition

    n_mega = r_core // (MEGA_BLK * 128)

    nc = bacc.Bacc(num_devices=n_cores)

    x_in = nc.declare_dram_parameter(
        "x", [n_mega, 128, MEGA_BLK * 64], F32, isOutput=False
    )
    m0_in = nc.declare_dram_parameter("m0", [r_core], F32, isOutput=False)
    m1_in = nc.declare_dram_parameter("m1", [r_core], F32, isOutput=False)
    w1t_in = nc.declare_dram_parameter("w1t", [IN_DIM, HID], F16, isOutput=False)
    w2s_in = nc.declare_dram_parameter("w2s", [HID, 32 * 32], F16, isOutput=False)
    b1_in = nc.declare_dram_parameter("b1", [HID], F32, isOutput=False)
    b2_in = nc.declare_dram_parameter("b2", [1], F32, isOutput=False)
    out_t = nc.declare_dram_parameter("out", [r_core], F32, isOutput=True)
    gs_t = nc.declare_dram_parameter("gsums", [2], F32, isOutput=True)

    cc_in = nc.dram_tensor("cc_in", [2], F32)
    cc_out = nc.dram_tensor("cc_out", [2], F32, addr_space="Shared")
    cc_warm_in = nc.dram_tensor("cc_warm_in", [2], F32)
    cc_warm_out = nc.dram_tensor("cc_warm_out", [2], F32, addr_space="Shared")

    B_ELEM = IN_DIM * 128  # elements per x block

    with ExitStack() as ctx:
        tc = ctx.enter_context(tile.TileContext(nc))
        singles = ctx.enter_context(tc.tile_pool(name="singles", bufs=1))
        xx_pool = ctx.enter_context(tc.tile_pool(name="xx", bufs=3))
        ht_pool = ctx.enter_context(tc.tile_pool(name="ht", bufs=3))
        ph_pool = ctx.enter_context(tc.tile_pool(name="ph", bufs=3, space="PSUM"))
        ps_pool = ctx.enter_context(tc.tile_pool(name="ps", bufs=1, space="PSUM"))

        # ---- static setup ----------------------------------------------
        w1t_sb = singles.tile([128, HID], F16)     # both halves hold W1T
        nc.sync.dma_start(
            out=w1t_sb[:],
            in_=_ap(w1t_in[:], 0, [[0, 2], [HID, IN_DIM], [1, HID]]),
        )
        b1_sb = singles.tile([128, 1], F32)
        nc.sync.dma_start(out=b1_sb[:], in_=_ap(b1_in[:], 0, [[1, HID], [1, 1]]))
        b2_sb = singles.tile([128, 1], F32)
        nc.sync.dma_start(out=b2_sb[:], in_=_ap(b2_in[:], 0, [[0, 128], [1, 1]]))

        # Warmup collective: absorbs ncfw cold-start + inter-core launch
        # skew during the compute phase, so the real AllReduce at the end
        # runs at the warm floor.
        warm_src = singles.tile([128, 2], F32)
        nc.vector.memset(warm_src[:], 0.0)
        if use_coll:
            nc.gpsimd.dma_start(out=cc_warm_in[:], in_=warm_src[0:1, :])
            nc.gpsimd.collective_compute(
                "AllReduce",
                ALU.add,
                replica_groups=[list(range(n_cores))],
                ins=[cc_warm_in[:]],
                outs=[cc_warm_out[:]],
            )

        # 32 strip matrices [128, 32] fp16, strip c has W2 in column c.
        strips = singles.tile([128, 32, 32], F16)
        nc.sync.dma_start(
            out=strips[:], in_=_ap(w2s_in[:], 0, [[32 * 32, HID], [1, 32 * 32]])
        )

        # Masks + persistent E (all f32, SBUF-resident for the whole kernel).
        m0_sb = singles.tile([128, cols], F32)
        m1_sb = singles.tile([128, cols], F32)
        mask_dims = [[Q, 128], [128 * Q, n_st], [1, Q]]
        nc.sync.dma_start(out=m0_sb[:], in_=_ap(m0_in[:], 0, mask_dims))
        nc.sync.dma_start(out=m1_sb[:], in_=_ap(m1_in[:], 0, mask_dims))
        e_sb = singles.tile([128, cols], F32)
        scratch = singles.tile([128, cols], F32)
        out_sb = singles.tile([128, cols], F32)
        rr = singles.tile([128, 2], F32)
        rr_red = singles.tile([128, 2], F32)
        ones_sb = singles.tile([128, 1], F32)
        nc.vector.memset(ones_sb[:], 1.0)
        g_sb = singles.tile([128, 2], F32)
        inv = singles.tile([128, 2], F32)
        dinv = singles.tile([128, 1], F32)

        # ---- phase 1: matmuls / tanh / scores / exp --------------------
        for st in range(n_st):
            s_ps = ps_pool.tile([128, Q], F32, tag="score")
            for m in range(_MEGAS_PER_ST):
                mega = st * _MEGAS_PER_ST + m
                half = MEGA_BLK * 64  # 2048 rows: partition halves g=0/1
                xx = xx_pool.tile([128, half], F16, tag="xx")
                src = _ap(
                    x_in[:],
                    mega * 128 * half,
                    [[half, 128], [1, half]],
                )
                nc.gpsimd.dma_start(out=xx[:], in_=src)  # f32 -> f16 cast DMA

                ht = ht_pool.tile([128, MEGA_BLK * 128], F16, tag="ht")
                for t in range(_SUB_PER_MEGA):
                    ph = ph_pool.tile([128, 1024], F32, tag="ph")
                    nc.tensor.matmul(
                        ph[:, 0:512],
                        w1t_sb[0:64, :],
                        xx[0:64, t * 512 : (t + 1) * 512],
                        start=True,
                        stop=True,
                    )
                    nc.tensor.matmul(
                        ph[:, 512:1024],
                        w1t_sb[64:128, :],
                        xx[64:128, t * 512 : (t + 1) * 512],
                        start=True,
                        stop=True,
                    )
                    # ht col layout is (t, g, j): col = t*1024 + g*512 + j,
                    # holding row mega_base + g*2048 + t*512 + j.
                    nc.scalar.activation(
                        out=ht[:, t * 1024 : (t + 1) * 1024],
                        in_=ph[:, 0:1024],
                        func=ACTF.Tanh,
                        bias=b1_sb[:],
                        scale=1.0,
                    )
                for bl in range(_BLOCKS_PER_MEGA):
                    b = m * _BLOCKS_PER_MEGA + bl
                    c = b % 32
                    g = b // 32
                    # rows bl*256..+256 of this mega live at ht col offset:
                    hoff = ((bl % 8) // 2) * 1024 + (bl // 8) * 512 + (bl % 2) * Q
                    nc.tensor.matmul(
                        s_ps[32 * g : 32 * g + 32, :],
                        strips[:, c, :],
                        ht[:, hoff : hoff + Q],
                        start=(c == 0),
                        stop=(c == 31),
                        skip_group_check=True,
                        tile_position=(0, 32 * g),
                    )
            nc.scalar.activation(
                out=e_sb[:, st * Q : (st + 1) * Q],
                in_=s_ps[:],
                func=ACTF.Exp,
                bias=b2_sb[:],
                scale=1.0,
            )

        # ---- segment sums + allreduce ----------------------------------
        if stage <= 1:
            # phase-1 only: dump E and a dummy gsums
            nc.sync.dma_start(
                out=_ap(out_t[:], 0, [[Q, 128], [128 * Q, n_st], [1, Q]]),
                in_=e_sb[:],
            )
            nc.sync.dma_start(out=gs_t[:], in_=e_sb[0:1, 0:2])
            nc.compile()
            return nc
        nc.vector.tensor_mul(scratch[:], e_sb[:], m0_sb[:])
        nc.vector.reduce_sum(rr[:, 0:1], scratch[:], axis=mybir.AxisListType.X)
        nc.vector.tensor_mul(scratch[:], e_sb[:], m1_sb[:])
        nc.vector.reduce_sum(rr[:, 1:2], scratch[:], axis=mybir.AxisListType.X)
        if stage <= 2:
            # skip partition reduce: use per-partition sums (wrong values)
            nc.vector.tensor_copy(rr_red[:], rr[:])
        else:
            # cross-partition sum via ones-matmul (PE), [128,2] -> [1,2]
            ps_rr = ps_pool.tile([128, 2], F32, tag="score")
            nc.tensor.matmul(
                ps_rr[0:1, :], ones_sb[:], rr[:], start=True, stop=True
            )
            nc.scalar.activation(
                out=rr_red[0:1, :],
                in_=ps_rr[0:1, :],
                func=ACTF.Copy,
                bias=0.0,
                scale=1.0,
            )
        if use_coll:
            nc.gpsimd.dma_start(out=cc_in[:], in_=rr_red[0:1, :])
            nc.gpsimd.collective_compute(
                "AllReduce",
                ALU.add,
                replica_groups=[list(range(n_cores))],
                ins=[cc_in[:]],
                outs=[cc_out[:]],
            )
            nc.sync.dma_start(out=gs_t[:], in_=cc_out[:])
            nc.sync.dma_start(
                out=g_sb[:], in_=_ap(cc_out[:], 0, [[0, 128], [1, 2]])
            )
        else:
            nc.sync.dma_start(out=gs_t[:], in_=rr_red[0:1, :])
            nc.vector.tensor_copy(g_sb[:], rr_red[:])

        # ---- normalize + store -----------------------------------------
        nc.vector.reciprocal(out=inv[:], in_=g_sb[:])
        nc.vector.tensor_sub(dinv[:], inv[:, 1:2], inv[:, 0:1])
        nc.vector.tensor_scalar(
            out=scratch[:],
            in0=m1_sb[:],
            scalar1=dinv[:],
            scalar2=inv[:, 0:1],
            op0=ALU.mult,
            op1=ALU.add,
        )
        nc.vector.tensor_mul(out_sb[:], scratch[:], e_sb[:])
        nc.sync.dma_start(
            out=_ap(out_t[:], 0, [[Q, 128], [128 * Q, n_st], [1, Q]]),
            in_=out_sb[:],
        )

    nc.compile()
    return nc


_NC_CACHE = {}


def _get_nc(n_st=N_ST):
    if n_st not in _NC_CACHE:
        _NC_CACHE[n_st] = build_nc(n_st=n_st)
    return _NC_CACHE[n_st]


def prep_inputs(x, T, W1, b1, W2, b2, n_st=N_ST, n_cores=N_CORES):
    """Host-side shard/layout prep -> per-core input maps."""
    r_core = n_st * 128 * Q
    nblk = r_core // 128
    n_pad = n_cores * r_core
    n_rows = x.shape[0]
    nblk_real = n_rows // 128

    x = np.ascontiguousarray(np.asarray(x, dtype=np.float32))
    rows_mega = MEGA_BLK * 128                      # 4096
    half = rows_mega // 2                           # 2048
    n_mega_tot = n_pad // rows_mega
    n_full = n_rows // rows_mega
    xd = np.zeros((n_mega_tot, 128, half), dtype=np.float32)
    xd[:n_full] = (
        x[: n_full * rows_mega]
        .reshape(n_full, 2, half, IN_DIM)
        .transpose(0, 1, 3, 2)
        .reshape(n_full, 128, half)
    )
    rem = n_rows - n_full * rows_mega
    if rem:
        r0 = min(rem, half)
        xd[n_full, :IN_DIM, :r0] = x[n_full * rows_mega :][:r0].T
        if rem > half:
            xd[n_full, IN_DIM:, : rem - half] = x[n_full * rows_mega + half :].T
    n_mega_core = n_mega_tot // n_cores

    T = np.asarray(T)
    m0 = np.zeros(n_pad, dtype=np.float32)
    m1 = np.zeros(n_pad, dtype=np.float32)
    m0[:n_rows] = T == 0
    m1[:n_rows] = T == 1

    w1t = np.ascontiguousarray(np.asarray(W1, np.float32).T).astype(np.float16)
    w2s = np.zeros((HID, 32, 32), dtype=np.float16)
    w2v = np.asarray(W2, np.float32).reshape(HID).astype(np.float16)
    for c in range(32):
        w2s[:, c, c] = w2v
    w2s = w2s.reshape(HID, 32 * 32)
    b1h = np.asarray(b1, np.float32).reshape(HID).copy()
    b2h = np.asarray(b2, np.float32).reshape(1).copy()

    in_maps = []
    for cid in range(n_cores):
        in_maps.append(
            {
                "x": xd[cid * n_mega_core : (cid + 1) * n_mega_core],
                "m0": m0[cid * r_core : (cid + 1) * r_core],
                "m1": m1[cid * r_core : (cid + 1) * r_core],
                "w1t": w1t,
                "w2s": w2s,
                "b1": b1h,
                "b2": b2h,
            }
        )
    return in_maps


def run(x, T, W1, b1, W2, b2, n_st=N_ST, trace=False, trace_cores=None):
    in_maps = prep_inputs(x, T, W1, b1, W2, b2, n_st=n_st)
    nc = _get_nc(n_st)
    res = run_bass_kernel_spmd(
        nc, in_maps, list(range(N_CORES)), trace=trace, trace_cores=trace_cores
    )
    out = np.concatenate([res.results[c]["out"] for c in range(N_CORES)])
    return out[: x.shape[0]].astype(np.float32, copy=False), res


def kernel(x, T, W1, b1, W2, b2):
    out, _ = run(x, T, W1, b1, W2, b2)
    return out

